# revision 36
# baseline (speedup 1.0000x reference)
"""ViT attention block (B=64, N=197, H=12, hd=64, D=768) on 8 trn2 NeuronCores.

Pure data-parallel: 8 batches per core.  Head-PAIR layout throughout: heads
(2g, 2g+1) live on partition halves [0:64) / [64:128) of one 128-partition
tile, which keeps every matmul on the full PE array:

  xt   <- x pre-TRANSPOSED ON THE HOST (like the weights), loaded with
          plain token-chunk-major copies on the sync queue so the first
          qk n-tile's inputs land in ~2.5us; weight / const copies ride
          the gpsimd + ACT DMA queues concurrently
  q,k  <- W_qk @ xt   per-PAIR M=128 tiles -> qkT[128, 12, tok]
          (q pre-scaled by 1/8 on host, +q_bias folded into eviction;
          pairs 0-5 = q, 6-11 = k)
  v    <- xt.T @ W_v  (natural layout [tok, feat], per-batch M-tiles)
  per batch b, head-pair g:
    S       = k_h^T q_h, both heads CONCURRENT via PE row tiling
              (even: tile_position=(0,0), odd: (64,0)) into SEPARATE
              banks of one [128,1024] tile -- concurrent PE drains into
              a single PSUM bank crash this HW revision
    E       = exp(S) (ACT), e2 = E * exp_rpb (GPSIMD, host-precomputed)
    sums    = ones.T @ e2 (PE, M=128 -> replicated on all partitions)
    Bt      = 1/sums (DVE reciprocal_approx_fast, [128,394] per pair)
    O       = v_h^T e2 per head, CONCURRENT via PE col tiling into two
              banks (even -> psum[0:64] tp=(0,0), odd -> psum[64:128]
              tp=(0,64))
    outT    = O * Bt (DVE, pair-aligned slices) -> outT[128, 6, tok]
  y = outT.T @ proj_w (K=128 contraction per pair) + proj_b during
      eviction (v_bias pre-folded into proj_b: softmax rows sum to 1);
      y returned as bf16 (rel-err budget allows it; halves output DMA)

tile_position facts verified on this trn2 revision by probe.py:
upper-quadrant streaming (row tile (64,0)) and base-64 psum outputs
(col tile (0,64)) both work when tile_position is explicit; one PSUM
bank must never hold two CONCURRENT accumulation groups, and two
concurrent matmuls must never DRAIN into the same bank (single-mm
groups into one bank at different partition halves are fine).
Empty tile pools (opened but never .tile()d) crash at runtime - pool
scopes are gated behind the phase flags.  fp8 was evaluated and ruled
out numerically: qkv or proj in fp8e4m3 gives 3.4e-2..4.9e-2 max-norm
rel err vs the 2e-2 gate (bf16 everywhere: 5.5e-3 measured on HW).

CoreSim timeline: 313.8us (previous M=64 baseline) -> 167.3us; PE busy
150us at ~90% occupancy (host-side x transpose killed the DMA-transpose
startup stalls; outputs go out on the sync HWDGE queue).  The sim
serializes all matmuls and models a single global DMA device, so the PE
row/col-tile concurrency and multi-queue DMA overlap are NOT credited
in sim time - real HW should sit a further ~10% below the sim number.
"""

import os
import sys

import numpy as np

for _p in ("/opt/trn_rl_repo", os.path.expanduser("~/.axon_site/_ro/trn_rl_repo")):
    if os.path.isdir(_p) and _p not in sys.path:
        sys.path.insert(0, _p)

import ml_dtypes  # noqa: E402

B = 64
NTOK = 197
DIM = 768
HEADS = 12
HD = 64
NCORES = 8
BS = B // NCORES  # 8 batches per core
NT = BS * NTOK  # 1576 real tokens per core
NTP = 1600  # padded tokens (mult of 16 for xbar; 12x128 + 64)
SCALE = HD ** -0.5
NP = HEADS // 2  # 6 head pairs
W2 = 2 * NTOK  # 394

_CACHE = {}


def _build_bass():
    import concourse.mybir as mybir
    import concourse.tile as tile
    from concourse import bacc

    f32 = mybir.dt.float32
    bf16 = mybir.dt.bfloat16
    EXP = mybir.ActivationFunctionType.Exp

    nc = bacc.Bacc(
        "TRN2", target_bir_lowering=False, debug=False,
        num_devices=int(os.environ.get("K_NDEV", str(NCORES))),
    )

    x_d = nc.dram_tensor("x", [DIM, NTP], bf16, kind="ExternalInput")
    qkvw_d = nc.dram_tensor("qkv_wt", [DIM, 3 * DIM], bf16, kind="ExternalInput")
    qb_d = nc.dram_tensor("qb", [NP, 128, 1], f32, kind="ExternalInput")
    projw_d = nc.dram_tensor("proj_wt", [DIM, DIM], bf16, kind="ExternalInput")
    pb_d = nc.dram_tensor("pb", [128, DIM], bf16, kind="ExternalInput")
    rpb_d = nc.dram_tensor("exp_rpb", [NTOK, HEADS * NTOK], bf16, kind="ExternalInput")
    y_d = nc.dram_tensor("y", [NT, DIM], bf16, kind="ExternalOutput")

    # token n-tiles for qk phase
    NTILES = [(0, 512), (512, 512), (1024, 512), (1536, 64)]
    QKTILES = [(0, 512), (512, 512), (1024, 512), (1536, 40)]
    VTILES = [(0, 512), (512, 256)]

    with tile.TileContext(nc, linearize=bool(os.environ.get("K_LINEARIZE"))) as tc:
        with (
            tc.tile_pool(name="consts", bufs=1) as consts,
            tc.tile_pool(name="acts", bufs=1) as acts,
        ):
            # ---- constant + input loads ----
            projw = consts.tile([128, NP, DIM], bf16)
            rpb = consts.tile([128, 2, HEADS * NTOK], bf16)
            qb = consts.tile([128, NP, 1], f32)
            pb = consts.tile([128, DIM], bf16)
            ones = consts.tile([128, 128], bf16)

            # persistent activations (all in head-pair layout)
            qkT = acts.tile([128, 2 * NP, NTP], bf16)  # pairs 0-5 q, 6-11 k
            vsb = acts.tile([128, 2 * BS, DIM], bf16)  # v natural, per (b, chunk)
            outT = acts.tile([128, NP, NTP], bf16)  # attn out, pair layout

            # All plain copies go on the ACT queue; the sync queue carries
            # ONLY the x transposes (concurrent queues verified by probe2).
            nc.scalar.dma_start(out=qb[:, :, :], in_=qb_d[:].rearrange("k p o -> p k o"))
            nc.vector.memset(ones[:, :], 1.0)
            # proj reads padded token cols 1576:1600; keep them finite
            if os.environ.get("K_FULLMEMSET"):
                nc.vector.memset(outT[:, :, :], 0.0)
            else:
                nc.vector.memset(outT[:, :, NT:NTP], 0.0)

            stop_after = os.environ.get("K_STOP_AFTER", "")
            SUB = os.environ.get("K_SUB", "all")
            sublv = {"scores": 0, "exp": 1, "mul": 2, "sums": 3, "recip": 4,
                     "av": 5, "all": 9}[SUB]
            do_qkv = stop_after != "load"
            do_attn = do_qkv and stop_after != "qkv"
            do_proj = do_attn and stop_after != "attn" and sublv >= 9

            # ---- qkv projections ----
            with (
                tc.tile_pool(name="ldp", bufs=1) as ldp,
                tc.tile_pool(name="ps_qk", bufs=4, space="PSUM") as ps_qk,
                tc.tile_pool(name="ps_v", bufs=2, space="PSUM") as ps_v,
            ):
                qkvw = ldp.tile([128, 6, 3 * DIM], bf16)
                xt = ldp.tile([128, 6, NTP], bf16)  # x transposed [c, tok]
                # x arrives pre-transposed from the host: plain copies on
                # the sync queue, token-chunk-major so the first qk n-tile's
                # inputs land ASAP
                x_v = x_d[:].rearrange("(k p) t -> p k t", p=128)
                for toff, tsz in NTILES:
                    for k in range(6):
                        nc.sync.dma_start(
                            out=xt[:, k, toff : toff + tsz],
                            in_=x_v[:, k, toff : toff + tsz],
                        )
                # qkvw on the gpsimd queue (idle until attention), q cols
                # first so the first m-tiles unblock ASAP
                qkvw_v = qkvw_d[:].rearrange("(k p) n -> p k n", p=128)
                for k in range(6):
                    nc.gpsimd.dma_start(
                        out=qkvw[:, k, 0:768], in_=qkvw_v[:, k, 0:768]
                    )
                for k in range(6):
                    nc.gpsimd.dma_start(
                        out=qkvw[:, k, 768:1536], in_=qkvw_v[:, k, 768:1536]
                    )

                for k in range(6):
                    nc.gpsimd.dma_start(
                        out=qkvw[:, k, 1536:2304], in_=qkvw_v[:, k, 1536:2304]
                    )
                # remaining consts (needed much later) trail on the ACT queue
                nc.scalar.dma_start(out=rpb[:, 0, :], in_=rpb_d[0:128, :])
                nc.scalar.dma_start(out=rpb[0:69, 1, :], in_=rpb_d[128:NTOK, :])
                projw_v = projw_d[:].rearrange("(g p) n -> p g n", p=128)
                for g in range(NP):
                    nc.scalar.dma_start(out=projw[:, g, :], in_=projw_v[:, g, :])
                nc.scalar.dma_start(out=pb[:, :], in_=pb_d[:, :])

                # n-outer; each batch's v is emitted as soon as its tokens
                # are projected, so attention b0 unblocks much earlier
                done_b = 0
                for noff, nsz in (QKTILES if do_qkv else []):
                    for m in range(2 * NP):
                        ps = ps_qk.tile([128, 512], f32)
                        for k in range(6):
                            nc.tensor.matmul(
                                ps[:, :nsz],
                                qkvw[:, k, m * 128 : (m + 1) * 128],
                                xt[:, k, noff : noff + nsz],
                                start=(k == 0),
                                stop=(k == 5),
                            )
                        if m < NP:  # q: add bias (pre-scaled on host)
                            nc.vector.tensor_scalar_add(
                                qkT[:, m, noff : noff + nsz],
                                ps[:, :nsz],
                                qb[:, m, 0:1],
                            )
                        else:  # k: plain copy
                            nc.scalar.copy(qkT[:, m, noff : noff + nsz], ps[:, :nsz])
                    tok_ready = noff + nsz
                    while done_b < BS and (done_b + 1) * NTOK <= tok_ready:
                        for mc in range(2):
                            msz = 128 if mc == 0 else NTOK - 128
                            toff = done_b * NTOK + mc * 128
                            psv = ps_v.tile([128, DIM], f32)
                            for k in range(6):
                                for vnoff, vnsz in VTILES:
                                    nc.tensor.matmul(
                                        psv[:msz, vnoff : vnoff + vnsz],
                                        xt[:, k, toff : toff + msz],
                                        qkvw[:, k, 1536 + vnoff : 1536 + vnoff + vnsz],
                                        start=(k == 0),
                                        stop=(k == 5),
                                    )
                            nc.scalar.copy(
                                vsb[:msz, done_b * 2 + mc, :], psv[:msz, :]
                            )
                        done_b += 1

            # ---- attention (head-pair concurrent via PE tiling) ----
            if do_attn:
             with (
                tc.tile_pool(name="work", bufs=3) as work,
                tc.tile_pool(name="e2p", bufs=2) as e2p,
                tc.tile_pool(name="bp", bufs=2) as bp,
                tc.tile_pool(name="ps_s", bufs=2, space="PSUM") as ps_s,
                tc.tile_pool(name="ps_sum", bufs=2, space="PSUM") as ps_sum,
                tc.tile_pool(name="ps_o", bufs=1, space="PSUM") as ps_o,
            ):
                for b in range(BS):
                    tb = b * NTOK
                    e2 = e2p.tile([128, 2, HEADS * NTOK], bf16)
                    Bt = bp.tile([128, NP * W2], f32)
                    for g in range(NP):
                        sm = ps_sum.tile([128, 512], f32)
                        for mc in range(2):
                            msz = 128 if mc == 0 else NTOK - 128
                            ts = tb + mc * 128
                            # both heads' scores concurrently (row tiling);
                            # concurrent PE drains need SEPARATE banks
                            S = ps_s.tile([128, 1024], f32)
                            nc.tensor.matmul(
                                S[:msz, 0:NTOK],
                                qkT[0:64, NP + g, ts : ts + msz],
                                qkT[0:64, g, tb : tb + NTOK],
                                start=True, stop=True,
                                tile_position=(0, 0),
                            )
                            nc.tensor.matmul(
                                S[:msz, 512 : 512 + NTOK],
                                qkT[64:128, NP + g, ts : ts + msz],
                                qkT[64:128, g, tb : tb + NTOK],
                                start=True, stop=True,
                                tile_position=(64, 0),
                            )
                            if sublv < 1:
                                continue
                            exps = work.tile([128, W2], bf16)
                            nc.scalar.activation(
                                exps[:msz, :].rearrange("p (s n) -> p s n", s=2),
                                S[:msz, :].rearrange("p (s n) -> p s n", s=2)[:, :, :NTOK],
                                EXP,
                            )
                            if sublv < 2:
                                continue
                            nc.gpsimd.tensor_mul(
                                e2[:msz, mc, g * W2 : (g + 1) * W2],
                                exps[:msz, :],
                                rpb[:msz, mc, g * W2 : (g + 1) * W2],
                            )
                            if sublv < 3:
                                continue
                            # column sums of both heads, replicated across
                            # all 128 partitions
                            nc.tensor.matmul(
                                sm[:, 0:W2],
                                ones[:msz, :],
                                e2[:msz, mc, g * W2 : (g + 1) * W2],
                                start=(mc == 0),
                                stop=(mc == 1),
                            )
                        if sublv < 4:
                            continue
                        nc.vector.reciprocal_approx_fast(
                            out=Bt[:, g * W2 : (g + 1) * W2],
                            in_=sm[:, 0:W2],
                        )
                        if sublv < 5:
                            continue
                        # AV: both heads concurrently (col tiling), separate
                        # banks (2-mm accumulation groups)
                        Oe = ps_o.tile([128, 512], f32)
                        Oo = ps_o.tile([128, 512], f32)
                        for mc in range(2):
                            msz = 128 if mc == 0 else NTOK - 128
                            nc.tensor.matmul(
                                Oe[0:64, 0:NTOK],
                                vsb[:msz, b * 2 + mc, (2 * g) * 64 : (2 * g + 1) * 64],
                                e2[:msz, mc, (2 * g) * NTOK : (2 * g + 1) * NTOK],
                                start=(mc == 0), stop=(mc == 1),
                                tile_position=(0, 0),
                            )
                            nc.tensor.matmul(
                                Oo[64:128, 0:NTOK],
                                vsb[:msz, b * 2 + mc, (2 * g + 1) * 64 : (2 * g + 2) * 64],
                                e2[:msz, mc, (2 * g + 1) * NTOK : (2 * g + 2) * NTOK],
                                start=(mc == 0), stop=(mc == 1),
                                tile_position=(0, 64),
                            )
                        if sublv < 6:
                            continue
                        nc.vector.tensor_mul(
                            outT[0:64, g, tb : tb + NTOK],
                            Oe[0:64, 0:NTOK],
                            Bt[0:64, g * W2 : g * W2 + NTOK],
                        )
                        nc.vector.tensor_mul(
                            outT[64:128, g, tb : tb + NTOK],
                            Oo[64:128, 0:NTOK],
                            Bt[64:128, g * W2 + NTOK : (g + 1) * W2],
                        )

            if do_qkv and not do_proj and not os.environ.get("K_NOPROBE"):
                nc.gpsimd.dma_start(out=y_d[0:128, :], in_=qkT[:, 0, 0:DIM])
                nc.gpsimd.dma_start(out=y_d[128:256, :], in_=vsb[:, 0, :])
                if do_attn and sublv >= 6:
                    nc.gpsimd.dma_start(out=y_d[256:384, :], in_=outT[:, 0, 0:DIM])

            # ---- output projection (K=128 contraction per head pair) ----
            if do_proj:
             with (
                tc.tile_pool(name="yp", bufs=2) as yp,
                tc.tile_pool(name="ps_y", bufs=2, space="PSUM") as ps_y,
             ):
                for m in range(13):
                    moff = m * 128
                    msz = min(128, NTP - moff)
                    real = min(128, NT - moff)
                    Y = ps_y.tile([128, DIM], f32)
                    for noff, nsz in VTILES:
                        for g in range(NP):
                            nc.tensor.matmul(
                                Y[:msz, noff : noff + nsz],
                                outT[:, g, moff : moff + msz],
                                projw[:, g, noff : noff + nsz],
                                start=(g == 0),
                                stop=(g == NP - 1),
                            )
                    ysb = yp.tile([128, DIM], bf16)
                    nc.vector.tensor_add(ysb[:msz, :], Y[:msz, :], pb[:msz, :])
                    nc.sync.dma_start(out=y_d[moff : moff + real, :], in_=ysb[:real, :])

    nc.compile()
    return nc


def _prep_inputs(x, qkv_w, q_bias, v_bias, rpb_table, proj_w, proj_b, rel_pos_index):
    bf16 = ml_dtypes.bfloat16
    x = np.asarray(x, np.float32)
    qkv_w = np.asarray(qkv_w, np.float32)
    q_bias = np.asarray(q_bias, np.float32)
    v_bias = np.asarray(v_bias, np.float32)
    rpb_table = np.asarray(rpb_table, np.float32)
    proj_w = np.asarray(proj_w, np.float32)
    proj_b = np.asarray(proj_b, np.float32)
    rel_pos_index = np.asarray(rel_pos_index)

    qkv_wt = qkv_w.T.copy()  # [768, 2304]
    qkv_wt[:, :DIM] *= SCALE
    qkv_wt = np.ascontiguousarray(qkv_wt, dtype=bf16)

    qb = (q_bias * SCALE).reshape(NP, 128, 1).astype(np.float32)

    proj_wt = np.ascontiguousarray(proj_w.T, dtype=bf16)
    pb_eff = np.tile((proj_b + proj_w @ v_bias).reshape(1, DIM), (128, 1)).astype(bf16)

    # bias[h, n, m] = rpb_table[rel_pos_index[n, m], h]; store exp() as
    # [m-chunk, m-in-chunk, h*197 + n]
    bias_nmh = rpb_table[rel_pos_index]  # [n, m, h]
    er = np.exp(bias_nmh.transpose(1, 2, 0))  # [m, h, n]
    er = er.reshape(NTOK, HEADS * NTOK)
    exp_rpb = np.ascontiguousarray(er, dtype=bf16)

    shared = {
        "qkv_wt": qkv_wt,
        "qb": qb,
        "proj_wt": proj_wt,
        "pb": pb_eff,
        "exp_rpb": exp_rpb,
    }
    in_maps = []
    for c in range(NCORES):
        xc = x[c * BS : (c + 1) * BS].reshape(NT, DIM)
        xp = np.zeros((DIM, NTP), bf16)
        xp[:, :NT] = np.ascontiguousarray(xc.T).astype(bf16)
        in_maps.append({"x": xp, **shared})
    return in_maps


def run(inputs, trace=False):
    """Build (cached), run on 8 cores, return (y_full, BassKernelResults)."""
    from concourse.bass_utils import run_bass_kernel_spmd

    if "nc" not in _CACHE:
        _CACHE["nc"] = _build_bass()
    nc = _CACHE["nc"]
    in_maps = _prep_inputs(**{k: inputs[k] for k in (
        "x", "qkv_w", "q_bias", "v_bias", "rpb_table", "proj_w", "proj_b",
        "rel_pos_index")})
    try:
        res = run_bass_kernel_spmd(
            nc, in_maps, core_ids=list(range(NCORES)), trace=trace
        )
    except ModuleNotFoundError:
        # NTFF profile hook unavailable in this container; run untraced
        res = run_bass_kernel_spmd(
            nc, in_maps, core_ids=list(range(NCORES)), trace=False
        )
    y = np.concatenate(
        [res.results[c]["y"].astype(np.float32).reshape(BS, NTOK, DIM)
         for c in range(NCORES)], axis=0
    )
    return np.ascontiguousarray(y, np.float32), res


def kernel(**inputs) -> np.ndarray:
    y, _ = run(inputs, trace=False)
    return y
